# revision 1
# baseline (speedup 1.0000x reference)
"""Dynamic per-sample CNN (nn_ConvFunc) Trainium2 Bass kernel.

Reference computation (per sample b):
  cnn_inp = proj_w @ cat(lhs, rhs) + proj_b          # 1x1 conv, [128, 32, 32]
  out     = conv3x3(cnn_inp, W_b) + bias_b           # W_b, bias_b unpacked from question_rep[b]

Sharding: pure data parallel, 8 samples per NeuronCore (batch 64 / 8 cores).

Per-core device kernel, per sample:
  - proj: per 512-col half of the 32x32 pixel space, 2 accumulating fp32r
    matmuls (lhs-channels, rhs-channels) into one PSUM bank; DVE evicts PSUM
    (+proj_b per-partition bias) into the interior of a zeroed [128,34,34]
    padded SBUF tile;
  - conv: 9 taps x 2 halves of accumulating fp32r matmuls, tap-outer so
    consecutive matmuls share the stationary weights; rhs = shifted 16x32
    window of the padded tile (strided AP); DVE evicts PSUM (+cnn bias) to
    SBUF, one store DMA per sample.

fp32r notes: the PE runs 4-byte fp32r at bf16 rate (1 cycle/row for N>=512)
keeping 11 mantissa bits. Inputs are pre-rounded on host exactly like the HW
DMA cast (RNE to 11 mantissa bits) so plain HWDGE sync DMAs can be used.
A few dummy warmup matmuls run during the DMA ramp to lift the HAM clock gate.
"""

import numpy as np

import concourse.bass as bass
import concourse.mybir as mybir
from concourse import bacc
from concourse.tile import TileContext
from concourse.bass_utils import run_bass_kernel_spmd

# Problem shapes (hardcoded per contract)
B = 64
DIM = 128
H = W = 32
K = 3
KK = K * K
HW = H * W             # 1024
WDIM = DIM * DIM * KK  # 147456
NCORES = 8
SPC = B // NCORES      # samples per core
HP, WP = H + 2, W + 2  # padded 34x34
HALF = HW // 2         # 512 columns per PSUM bank
HROWS = H // 2         # 16 output rows per half
CST = 2 * DIM + SPC + 1  # const cols: [pw0 | pw1 | qb | pb]

FP = mybir.dt.float32
FR = mybir.dt.float32r

_BUILT = {}


def _round_fp32r(a):
    """Round fp32 to fp32r (11 mantissa bits, RNE) exactly like the HW DMA cast."""
    b = np.ascontiguousarray(a, dtype=np.float32).view(np.uint32)
    lsb = (b >> 12) & 1
    return ((b + 0x7FF + lsb) & 0xFFFFF000).view(np.float32)


def build_nc(mm_dt=FR, tap_outer=False):
    nc = bacc.Bacc("TRN2", target_bir_lowering=False, debug=False,
                   num_devices=NCORES)

    qw = nc.declare_dram_parameter("qw", [SPC, DIM, KK * DIM], FR, isOutput=False)
    xl = nc.declare_dram_parameter("xl", [SPC, DIM, HW], FR, isOutput=False)
    xr = nc.declare_dram_parameter("xr", [SPC, DIM, HW], FR, isOutput=False)
    cst = nc.declare_dram_parameter("cst", [DIM, CST], FR, isOutput=False)
    out = nc.declare_dram_parameter("out", [SPC, DIM, HW], FP, isOutput=True)

    with TileContext(nc) as tc:
        with (
            tc.tile_pool(name="const", bufs=1) as cpool,
            tc.tile_pool(name="wpool", bufs=4) as wpool,
            tc.tile_pool(name="xpool", bufs=4) as xpool,
            tc.tile_pool(name="xppool", bufs=4) as xppool,
            tc.tile_pool(name="opool", bufs=4) as opool,
            tc.tile_pool(name="pp_pool", bufs=2, space="PSUM") as pp_pool,
            tc.tile_pool(name="pc_pool", bufs=6, space="PSUM") as pc_pool,
        ):
            cst_sb = cpool.tile([DIM, CST], mm_dt)
            nc.sync.dma_start(out=cst_sb[:], in_=cst[:])
            pw0 = cst_sb[:, 0:DIM]
            pw1 = cst_sb[:, DIM:2 * DIM]

            def qb_ap(s):
                return cst_sb[:, 2 * DIM + s:2 * DIM + s + 1].bitcast(FP)

            pb_ap = cst_sb[:, 2 * DIM + SPC:2 * DIM + SPC + 1].bitcast(FP)

            def proj(s):
                xl_sb = xpool.tile([DIM, HW], mm_dt, tag="xl")
                xr_sb = xpool.tile([DIM, HW], mm_dt, tag="xr")
                if s == 0:
                    # sample 0 is latency-critical: land operands in the order
                    # the first matmuls need them
                    for h in range(2):
                        nc.sync.dma_start(out=xl_sb[:, h * HALF:(h + 1) * HALF],
                                          in_=xl[s, :, h * HALF:(h + 1) * HALF])
                        nc.sync.dma_start(out=xr_sb[:, h * HALF:(h + 1) * HALF],
                                          in_=xr[s, :, h * HALF:(h + 1) * HALF])
                else:
                    nc.sync.dma_start(out=xl_sb[:], in_=xl[s])
                    nc.sync.dma_start(out=xr_sb[:], in_=xr[s])
                xp = xppool.tile([DIM, HP, WP], mm_dt, tag="xp")
                nc.vector.memset(xp[:, 0:1, :].bitcast(FP), 0.0)
                nc.vector.memset(xp[:, HP - 1:HP, :].bitcast(FP), 0.0)
                nc.vector.memset(xp[:, 1:HP - 1, 0:1].bitcast(FP), 0.0)
                nc.vector.memset(xp[:, 1:HP - 1, WP - 1:WP].bitcast(FP), 0.0)
                for h in range(2):
                    ppt = pp_pool.tile([DIM, HALF], FP, tag="pp")
                    nc.tensor.matmul(ppt[:], lhsT=pw0,
                                     rhs=xl_sb[:, h * HALF:(h + 1) * HALF],
                                     start=True, stop=False)
                    nc.tensor.matmul(ppt[:], lhsT=pw1,
                                     rhs=xr_sb[:, h * HALF:(h + 1) * HALF],
                                     start=False, stop=True)
                    nc.scalar.activation(
                        xp[:, 1 + HROWS * h:1 + HROWS * (h + 1), 1:1 + W],
                        ppt[:].rearrange("p (a b) -> p a b", b=W),
                        mybir.ActivationFunctionType.Identity,
                        bias=pb_ap,
                    )
                return xp

            def load_w(s):
                w_sb = wpool.tile([DIM, KK, DIM], mm_dt, tag="w")
                if s <= 1:
                    for t0, t1 in ((0, 3), (3, 6), (6, KK)):
                        nc.sync.dma_start(out=w_sb[:, t0:t1, :],
                                            in_=qw[s, :, t0 * DIM:t1 * DIM])
                else:
                    nc.sync.dma_start(out=w_sb[:], in_=qw[s])
                return w_sb

            def conv(s, xp, w_sb):
                o_sb = opool.tile([DIM, HW], FP, tag="o")
                pct0 = pc_pool.tile([DIM, HALF], FP, tag="pc")
                pct1 = pc_pool.tile([DIM, HALF], FP, tag="pc")
                pcts = [pct0, pct1]
                if tap_outer:
                    # consecutive matmuls share lhsT (same tap, both halves)
                    for t in range(KK):
                        kh, kw = divmod(t, K)
                        for h in range(2):
                            nc.tensor.matmul(
                                pcts[h][:],
                                lhsT=w_sb[:, t, :],
                                rhs=xp[:, HROWS * h + kh:HROWS * (h + 1) + kh,
                                       kw:kw + W],
                                start=(t == 0), stop=(t == KK - 1))
                else:
                    for h in range(2):
                        for t in range(KK):
                            kh, kw = divmod(t, K)
                            nc.tensor.matmul(
                                pcts[h][:],
                                lhsT=w_sb[:, t, :],
                                rhs=xp[:, HROWS * h + kh:HROWS * (h + 1) + kh,
                                       kw:kw + W],
                                start=(t == 0), stop=(t == KK - 1))
                for h in range(2):
                    nc.vector.tensor_scalar_add(
                        o_sb[:, h * HALF:(h + 1) * HALF], pcts[h][:], qb_ap(s))
                    if s == SPC - 1:
                        nc.scalar.dma_start(
                            out=out[s, :, h * HALF:(h + 1) * HALF],
                            in_=o_sb[:, h * HALF:(h + 1) * HALF])
                if s != SPC - 1:
                    nc.scalar.dma_start(out=out[s], in_=o_sb[:])

            # software pipeline: proj(s) ahead of conv(s-1) keeps PE dense
            prev = None
            for s in range(SPC):
                xp = proj(s)
                w_sb = load_w(s)
                if prev is not None:
                    conv(*prev)
                prev = (s, xp, w_sb)
            conv(*prev)

    nc.compile()
    return nc


def _prep(question_rep, lhs_rep, rhs_rep, proj_w, proj_b):
    """Host-side shard + layout prep (cheap reshapes/transposes only)."""
    qr = np.ascontiguousarray(question_rep, dtype=np.float32)
    # conv weights: [B, o, i, kh, kw] -> [B, i, (kh kw), o] so each tap is a
    # ready lhsT [i, o] block and the per-sample weight DMA is contiguous
    qw = qr[:, :WDIM].reshape(B, DIM, DIM, K, K).transpose(0, 2, 3, 4, 1)
    qw = _round_fp32r(np.ascontiguousarray(qw)).reshape(B, DIM, KK * DIM)
    qb = np.ascontiguousarray(qr[:, WDIM:])             # [B, 128]
    xl = _round_fp32r(lhs_rep).reshape(B, DIM, HW)
    xr = _round_fp32r(rhs_rep).reshape(B, DIM, HW)
    pwt = _round_fp32r(np.asarray(proj_w, dtype=np.float32).T)  # [256, 128]
    pb = np.asarray(proj_b, dtype=np.float32).reshape(DIM, 1)

    in_maps = []
    for c in range(NCORES):
        sl = slice(c * SPC, (c + 1) * SPC)
        # [pw0 | pw1 | qb(8 cols) | pb]; qb/pb stay exact fp32 (bitcast later)
        cstm = np.concatenate([pwt[:DIM], pwt[DIM:], qb[sl].T, pb],
                              axis=1).astype(np.float32)
        in_maps.append({
            "qw": np.ascontiguousarray(qw[sl]),
            "xl": np.ascontiguousarray(xl[sl]),
            "xr": np.ascontiguousarray(xr[sl]),
            "cst": np.ascontiguousarray(cstm),
        })
    return in_maps


def kernel(question_rep, lhs_rep, rhs_rep, proj_w, proj_b, _run_kwargs=None):
    if "nc" not in _BUILT:
        _BUILT["nc"] = build_nc()
    nc = _BUILT["nc"]
    in_maps = _prep(question_rep, lhs_rep, rhs_rep, proj_w, proj_b)
    res = run_bass_kernel_spmd(nc, in_maps, core_ids=list(range(NCORES)),
                               **(_run_kwargs or {}))
    out = np.concatenate([res.results[c]["out"] for c in range(NCORES)], axis=0)
    if _run_kwargs is not None:
        _BUILT["last_result"] = res
    return out.reshape(B, DIM, H, W)


if __name__ == "__main__":
    rng = np.random.default_rng(0)
    inputs = {
        "question_rep": rng.standard_normal((B, WDIM + DIM), dtype=np.float32) * 0.05,
        "lhs_rep": rng.standard_normal((B, DIM, H, W), dtype=np.float32),
        "rhs_rep": rng.standard_normal((B, DIM, H, W), dtype=np.float32),
        "proj_w": rng.standard_normal((DIM, 2 * DIM), dtype=np.float32),
        "proj_b": rng.standard_normal((DIM,), dtype=np.float32) * 0.01,
    }
    out = kernel(**inputs)
    print("ran, out shape:", out.shape)



# revision 3
# speedup vs baseline: 1.1198x; 1.1198x over previous
"""Dynamic per-sample CNN (nn_ConvFunc) Trainium2 Bass kernel.

Reference computation (per sample b):
  cnn_inp = proj_w @ cat(lhs, rhs) + proj_b          # 1x1 conv, [128, 32, 32]
  out     = conv3x3(cnn_inp, W_b) + bias_b           # W_b, bias_b unpacked from question_rep[b]

Sharding: pure data parallel, 8 samples per NeuronCore (batch 64 / 8 cores).

Per-core device kernel, per sample:
  - proj: per 512-col half of the 32x32 pixel space, 2 accumulating bf16
    matmuls (lhs-channels, rhs-channels) into one PSUM bank; ACT evicts PSUM
    (+proj_b per-partition bias) into the interior of a zero-bordered
    [128,34,34] padded SBUF tile;
  - conv: per half, 9 taps of accumulating bf16 matmuls; rhs = shifted 16x32
    window of the padded tile (strided AP); DVE evicts PSUM (+cnn bias) to
    SBUF, one store DMA per sample (split in halves for the last sample so
    the final store issues early).

All matmul operands are bf16 (rounded on host): the PE streams 1 col/cycle
either way, but bf16 halves HBM traffic and triggers FWL so LDWEIGHTS hides
behind the previous matmul. A few dummy warmup matmuls on a zeroed scratch
tile run during the DMA ramp to lift the HAM clock gate before real work.
"""

import numpy as np
import ml_dtypes

import concourse.bass as bass
import concourse.mybir as mybir
from concourse import bacc
from concourse.tile import TileContext
from concourse.bass_utils import run_bass_kernel_spmd

# Problem shapes (hardcoded per contract)
B = 64
DIM = 128
H = W = 32
K = 3
KK = K * K
HW = H * W             # 1024
WDIM = DIM * DIM * KK  # 147456
NCORES = 8
SPC = B // NCORES      # samples per core
HP, WP = H + 2, W + 2  # padded 34x34
HALF = HW // 2         # 512 columns per PSUM bank
HROWS = H // 2         # 16 output rows per half
NXP = 4                # persistent padded tiles (cycled s % NXP)
N_WARM = 5             # dummy warmup matmuls

FP = mybir.dt.float32
BF = mybir.dt.bfloat16
BF_NP = ml_dtypes.bfloat16

_BUILT = {}


def build_nc():
    nc = bacc.Bacc("TRN2", target_bir_lowering=False, debug=False,
                   num_devices=NCORES)

    qw = nc.declare_dram_parameter("qw", [SPC, DIM, KK * DIM], BF, isOutput=False)
    xl = nc.declare_dram_parameter("xl", [SPC, DIM, HW], BF, isOutput=False)
    xr = nc.declare_dram_parameter("xr", [SPC, DIM, HW], BF, isOutput=False)
    pw = nc.declare_dram_parameter("pw", [DIM, 2 * DIM], BF, isOutput=False)
    bia = nc.declare_dram_parameter("bia", [DIM, SPC + 1], FP, isOutput=False)
    out = nc.declare_dram_parameter("out", [SPC, DIM, HW], FP, isOutput=True)

    with TileContext(nc) as tc:
        with (
            tc.tile_pool(name="const", bufs=1) as cpool,
            tc.tile_pool(name="wpool", bufs=4) as wpool,
            tc.tile_pool(name="xpool", bufs=4) as xpool,
            tc.tile_pool(name="opool", bufs=4) as opool,
            tc.tile_pool(name="pd_pool", bufs=1, space="PSUM") as pd_pool,
            tc.tile_pool(name="pp_pool", bufs=2, space="PSUM") as pp_pool,
            tc.tile_pool(name="pc_pool", bufs=5, space="PSUM") as pc_pool,
        ):
            # --- warmup: dummy matmuls on a zeroed scratch tile keep the PE
            # busy through the DMA ramp so the HAM clock gate lifts early
            dummy = cpool.tile([DIM, HALF], BF)
            nc.vector.memset(dummy[:], 0.0)
            pdt = pd_pool.tile([DIM, HALF], FP)
            for _ in range(N_WARM):
                nc.tensor.matmul(pdt[:], lhsT=dummy[:, 0:DIM], rhs=dummy[:],
                                 start=True, stop=True)

            # --- constants: proj weights (bf16) + biases (fp32)
            pw_sb = cpool.tile([DIM, 2 * DIM], BF)
            nc.sync.dma_start(out=pw_sb[:], in_=pw[:])
            bia_sb = cpool.tile([DIM, SPC + 1], FP)
            nc.sync.dma_start(out=bia_sb[:], in_=bia[:])
            pw0 = pw_sb[:, 0:DIM]
            pw1 = pw_sb[:, DIM:2 * DIM]

            def qb_ap(s):
                return bia_sb[:, s:s + 1]

            pb_ap = bia_sb[:, SPC:SPC + 1]

            # --- persistent padded tiles: borders zeroed once, interiors
            # rewritten per sample (conv reads only rows/cols the proj wrote
            # plus the zero borders)
            xp_tiles = []
            for i in range(NXP):
                xp = cpool.tile([DIM, HP, WP], BF, tag=f"xp{i}")
                nc.vector.memset(xp[:, 0:1, :], 0.0)
                nc.vector.memset(xp[:, HP - 1:HP, :], 0.0)
                nc.vector.memset(xp[:, 1:HP - 1, 0:1], 0.0)
                nc.vector.memset(xp[:, 1:HP - 1, WP - 1:WP], 0.0)
                xp_tiles.append(xp)

            def proj(s):
                xl_sb = xpool.tile([DIM, HW], BF, tag="xl")
                xr_sb = xpool.tile([DIM, HW], BF, tag="xr")
                if s == 0:
                    # sample 0 is latency-critical: land operands in the order
                    # the first matmuls need them
                    for h in range(2):
                        nc.sync.dma_start(out=xl_sb[:, h * HALF:(h + 1) * HALF],
                                          in_=xl[s, :, h * HALF:(h + 1) * HALF])
                        nc.sync.dma_start(out=xr_sb[:, h * HALF:(h + 1) * HALF],
                                          in_=xr[s, :, h * HALF:(h + 1) * HALF])
                else:
                    nc.sync.dma_start(out=xl_sb[:], in_=xl[s])
                    nc.sync.dma_start(out=xr_sb[:], in_=xr[s])
                xp = xp_tiles[s % NXP]
                for h in range(2):
                    ppt = pp_pool.tile([DIM, HALF], FP, tag="pp")
                    nc.tensor.matmul(ppt[:], lhsT=pw0,
                                     rhs=xl_sb[:, h * HALF:(h + 1) * HALF],
                                     start=True, stop=False)
                    nc.tensor.matmul(ppt[:], lhsT=pw1,
                                     rhs=xr_sb[:, h * HALF:(h + 1) * HALF],
                                     start=False, stop=True)
                    nc.scalar.activation(
                        xp[:, 1 + HROWS * h:1 + HROWS * (h + 1), 1:1 + W],
                        ppt[:].rearrange("p (a b) -> p a b", b=W),
                        mybir.ActivationFunctionType.Identity,
                        bias=pb_ap,
                    )
                return xp

            def load_w(s):
                w_sb = wpool.tile([DIM, KK, DIM], BF, tag="w")
                if s <= 1:
                    for t0, t1 in ((0, 3), (3, 6), (6, KK)):
                        nc.sync.dma_start(out=w_sb[:, t0:t1, :],
                                          in_=qw[s, :, t0 * DIM:t1 * DIM])
                else:
                    nc.sync.dma_start(out=w_sb[:], in_=qw[s])
                return w_sb

            def conv(s, xp, w_sb):
                o_sb = opool.tile([DIM, HW], FP, tag="o")
                for h in range(2):
                    pct = pc_pool.tile([DIM, HALF], FP, tag="pc")
                    for t in range(KK):
                        kh, kw = divmod(t, K)
                        nc.tensor.matmul(
                            pct[:],
                            lhsT=w_sb[:, t, :],
                            rhs=xp[:, HROWS * h + kh:HROWS * (h + 1) + kh,
                                   kw:kw + W],
                            start=(t == 0), stop=(t == KK - 1))
                    nc.vector.tensor_scalar_add(
                        o_sb[:, h * HALF:(h + 1) * HALF], pct[:], qb_ap(s))
                    if s == SPC - 1:
                        # last sample: store halves as soon as they're ready
                        # so the final DMA issues early
                        nc.scalar.dma_start(
                            out=out[s, :, h * HALF:(h + 1) * HALF],
                            in_=o_sb[:, h * HALF:(h + 1) * HALF])
                if s != SPC - 1:
                    nc.scalar.dma_start(out=out[s], in_=o_sb[:])

            # software pipeline: proj(s) ahead of conv(s-1) keeps PE dense
            prev = None
            for s in range(SPC):
                xp = proj(s)
                w_sb = load_w(s)
                if prev is not None:
                    conv(*prev)
                prev = (s, xp, w_sb)
            conv(*prev)

    nc.compile()
    return nc


def _prep(question_rep, lhs_rep, rhs_rep, proj_w, proj_b):
    """Host-side shard + layout prep (cheap reshapes/casts only)."""
    qr = np.ascontiguousarray(question_rep, dtype=np.float32)
    # conv weights: [B, o, i, kh, kw] -> [B, i, (kh kw), o] so each tap is a
    # ready lhsT [i, o] block and the per-sample weight DMA is contiguous
    qw = qr[:, :WDIM].reshape(B, DIM, DIM, K, K).transpose(0, 2, 3, 4, 1)
    qw = np.ascontiguousarray(qw).astype(BF_NP).reshape(B, DIM, KK * DIM)
    qb = np.ascontiguousarray(qr[:, WDIM:])             # [B, 128] fp32
    xl = np.asarray(lhs_rep, dtype=np.float32).reshape(B, DIM, HW).astype(BF_NP)
    xr = np.asarray(rhs_rep, dtype=np.float32).reshape(B, DIM, HW).astype(BF_NP)
    pwt = np.asarray(proj_w, dtype=np.float32).T.astype(BF_NP)  # [256, 128]
    pwm = np.ascontiguousarray(
        np.concatenate([pwt[:DIM], pwt[DIM:]], axis=1))  # [128, 256]
    pb = np.asarray(proj_b, dtype=np.float32).reshape(DIM, 1)

    in_maps = []
    for c in range(NCORES):
        sl = slice(c * SPC, (c + 1) * SPC)
        biam = np.ascontiguousarray(
            np.concatenate([qb[sl].T, pb], axis=1), dtype=np.float32)
        in_maps.append({
            "qw": np.ascontiguousarray(qw[sl]),
            "xl": np.ascontiguousarray(xl[sl]),
            "xr": np.ascontiguousarray(xr[sl]),
            "pw": pwm,
            "bia": biam,
        })
    return in_maps


def kernel(question_rep, lhs_rep, rhs_rep, proj_w, proj_b, _run_kwargs=None):
    if "nc" not in _BUILT:
        _BUILT["nc"] = build_nc()
    nc = _BUILT["nc"]
    in_maps = _prep(question_rep, lhs_rep, rhs_rep, proj_w, proj_b)
    res = run_bass_kernel_spmd(nc, in_maps, core_ids=list(range(NCORES)),
                               **(_run_kwargs or {}))
    out = np.concatenate([res.results[c]["out"] for c in range(NCORES)], axis=0)
    if _run_kwargs is not None:
        _BUILT["last_result"] = res
    return out.reshape(B, DIM, H, W)


if __name__ == "__main__":
    rng = np.random.default_rng(0)
    inputs = {
        "question_rep": rng.standard_normal((B, WDIM + DIM), dtype=np.float32) * 0.05,
        "lhs_rep": rng.standard_normal((B, DIM, H, W), dtype=np.float32),
        "rhs_rep": rng.standard_normal((B, DIM, H, W), dtype=np.float32),
        "proj_w": rng.standard_normal((DIM, 2 * DIM), dtype=np.float32),
        "proj_b": rng.standard_normal((DIM,), dtype=np.float32) * 0.01,
    }
    out = kernel(**inputs)
    print("ran, out shape:", out.shape)
